# revision 5
# baseline (speedup 1.0000x reference)
"""Fused multi-head attention block (QKV + softmax + out-proj + residual + LayerNorm)
for Trainium2, SPMD over 8 NeuronCores.

Sharding: head-parallel. Core c owns heads {2c, 2c+1} for BOTH batch elements and
computes QKV projections + full attention for those heads. The per-head attention
outputs are exchanged with a single 8-way AllToAll so that core c ends up with all
1024 channels for its 512 output token rows (batch c//4, rows 512*(c%4)...). Each
core then does the output projection for its rows, residual add, and LayerNorm.

Layouts (host-prepared, all layout-only transforms):
  - xT[b]   = x[b].T as bf16, (1024 c, 2048 t): both matmul operands need the
              contraction dim (c) on partitions.
  - wqT/wkT/wvT = W[ch_local, :].T (1024 c, 128 d) bf16 where ch_local enumerates
              this core's 128 head-channels in (head, dim) order; note the
              reference splits features as (d_head, heads) with heads LAST, so
              head h's channels are {16*dd + h}.
  - woT     = Wo[:, ch_gath].T (1024, 1024) bf16 in the AllToAll's gathered
              channel order (src-core-major).
Attention per (batch, head): S^T tiles (tk on partitions) so softmax's sum over
keys is a matmul reduction: V is extended with a ones column (M=65 matmuls), whose
extra output row is the softmax denominator. exp() runs on the scalar engine with
the 1/sqrt(d_head) scale folded into the activation's free affine.
"""

import sys

sys.path.insert(0, "/opt/trn_rl_repo")

import numpy as np
import ml_dtypes

BF16 = ml_dtypes.bfloat16

B, T, D = 2, 2048, 1024
H, DH = 16, 64
N_CORES = 8
LN_EPS = 1e-5
HEADS_PER_CORE = 2
TROWS = T * B // N_CORES  # 512 output rows per core

_CACHE = {}


def _build():
    from contextlib import ExitStack
    import concourse.bass as bass
    import concourse.tile as tile
    from concourse import bacc, mybir

    f32 = mybir.dt.float32
    bf16 = mybir.dt.bfloat16
    AF = mybir.ActivationFunctionType
    ALU = mybir.AluOpType

    def bcast(ap_src, parts):
        """Broadcast a 1-D (or row) AP across `parts` partitions (step 0)."""
        return bass.AP(tensor=ap_src.tensor, offset=ap_src.offset,
                       ap=[[0, parts]] + [list(p) for p in ap_src.ap])

    nc = bacc.Bacc("TRN2", target_bir_lowering=False, debug=False,
                   num_devices=N_CORES)

    # ---- I/O ----
    xT_d = [nc.dram_tensor(f"xT{b}", [D, T], bf16, kind="ExternalInput")
            for b in range(B)]
    wqT_d = nc.dram_tensor("wqT", [D, 128], bf16, kind="ExternalInput")
    wkT_d = nc.dram_tensor("wkT", [D, 128], bf16, kind="ExternalInput")
    wvT_d = nc.dram_tensor("wvT", [D, 128], bf16, kind="ExternalInput")
    bq_d = nc.dram_tensor("bq", [128], f32, kind="ExternalInput")
    bk_d = nc.dram_tensor("bk", [128], f32, kind="ExternalInput")
    bv_d = nc.dram_tensor("bv", [128], f32, kind="ExternalInput")
    woT_d = nc.dram_tensor("woT", [D, D], bf16, kind="ExternalInput")
    bo_d = nc.dram_tensor("bo", [D], bf16, kind="ExternalInput")
    gamma_d = nc.dram_tensor("gamma", [D], f32, kind="ExternalInput")
    beta_d = nc.dram_tensor("beta", [D], f32, kind="ExternalInput")
    xres_d = nc.dram_tensor("xres", [TROWS, D], f32, kind="ExternalInput")
    out_d = nc.dram_tensor("out", [TROWS, D], f32, kind="ExternalOutput")

    NCH = 8  # 1024 / 128 contraction chunks

    with ExitStack() as ctx:
        tc = ctx.enter_context(tile.TileContext(nc))
        persist = ctx.enter_context(tc.tile_pool(name="persist", bufs=1))
        dram = ctx.enter_context(tc.tile_pool(name="dram", bufs=1, space="DRAM"))

        # ---- constants / weights into SBUF ----
        wqT_sb = persist.tile([128, NCH, 128], bf16)
        wkT_sb = persist.tile([128, NCH, 128], bf16)
        wvT_sb = persist.tile([128, NCH, 128], bf16)
        for w_sb, w_d in ((wqT_sb, wqT_d), (wkT_sb, wkT_d), (wvT_sb, wvT_d)):
            nc.sync.dma_start(
                out=w_sb[:],
                in_=w_d[:].rearrange("(ci p) d -> p ci d", p=128))
        woT_sb = persist.tile([128, NCH, D], bf16)
        for ci in range(NCH):
            nc.sync.dma_start(
                out=woT_sb[:, ci, :],
                in_=woT_d[128 * ci:128 * (ci + 1), :])
        bq_sb = persist.tile([128, 1], f32)
        bk_sb = persist.tile([128, 1], f32)
        nc.sync.dma_start(out=bq_sb[:], in_=bq_d[:].rearrange("(p f) -> p f", f=1))
        nc.sync.dma_start(out=bk_sb[:], in_=bk_d[:].rearrange("(p f) -> p f", f=1))
        bvb_sb = persist.tile([128, 128], f32)  # bv broadcast across partitions
        nc.sync.dma_start(out=bvb_sb[:], in_=bcast(bv_d[:], 128))
        bo_sb = persist.tile([1, D], bf16)
        nc.sync.dma_start(out=bo_sb[:], in_=bo_d[:].rearrange("(p f) -> p f", p=1))
        ones_sb = persist.tile([1, 128], bf16)
        nc.vector.memset(ones_sb[:], 1.0)
        gamma_sb = persist.tile([128, D], f32)
        beta_sb = persist.tile([128, D], f32)
        nc.sync.dma_start(out=gamma_sb[:], in_=bcast(gamma_d[:], 128))
        nc.sync.dma_start(out=beta_sb[:], in_=bcast(beta_d[:], 128))
        eps_sb = persist.tile([128, 1], f32)
        nc.vector.memset(eps_sb[:], LN_EPS)
        xres_sb = persist.tile([128, 4, D], f32)
        for mt in range(4):
            nc.sync.dma_start(out=xres_sb[:, mt, :],
                              in_=xres_d[128 * mt:128 * (mt + 1), :])

        # per-batch activation tensors
        xT_pool = ctx.enter_context(tc.tile_pool(name="xT", bufs=2))
        QT_sb = [persist.tile([128, T], bf16, name=f"QT{b}") for b in range(B)]
        KT_sb = [persist.tile([128, T], bf16, name=f"KT{b}") for b in range(B)]
        # V with ones column per head: [t-tile][128, head, 65]
        V_sb = [[persist.tile([128, HEADS_PER_CORE, DH + 1], bf16,
                              name=f"V{b}_{tt}") for tt in range(16)]
                for b in range(B)]
        for b in range(B):
            for tt in range(16):
                nc.vector.memset(V_sb[b][tt][:, :, DH:DH + 1], 1.0)

        # ---- phase 1: QKV projections (both batches) ----
        for b in range(B):
            xT_sb = xT_pool.tile([128, NCH, T], bf16, tag="xT")
            for ci in range(NCH):
                nc.sync.dma_start(out=xT_sb[:, ci, :],
                                  in_=xT_d[b][128 * ci:128 * (ci + 1), :])

            with tc.tile_pool(name=f"qkvps{b}", bufs=4, space="PSUM") as qkps:
                # Q^T, K^T: (128 ch, 2048 t), 4 n-chunks of 512
                for w_sb, bias_sb, dst in ((wqT_sb, bq_sb, QT_sb[b]),
                                           (wkT_sb, bk_sb, KT_sb[b])):
                    for n in range(4):
                        ps = qkps.tile([128, 512], f32, tag="qk")
                        for ci in range(NCH):
                            nc.tensor.matmul(
                                ps[:], w_sb[:, ci, :],
                                xT_sb[:, ci, 512 * n:512 * (n + 1)],
                                start=(ci == 0), stop=(ci == NCH - 1))
                        nc.scalar.add(dst[:, 512 * n:512 * (n + 1)], ps[:],
                                      bias_sb[:])
                # V natural: 16 t-tiles of (128 t, 128 ch)
                for tt in range(16):
                    ps = qkps.tile([128, 128], f32, tag="v")
                    for ci in range(NCH):
                        nc.tensor.matmul(
                            ps[:], xT_sb[:, ci, 128 * tt:128 * (tt + 1)],
                            wvT_sb[:, ci, :],
                            start=(ci == 0), stop=(ci == NCH - 1))
                    nc.vector.tensor_add(
                        V_sb[b][tt][:, :, 0:DH],
                        ps[:].rearrange("p (h d) -> p h d", h=HEADS_PER_CORE),
                        bvb_sb[:].rearrange("p (h d) -> p h d",
                                            h=HEADS_PER_CORE))

        # ---- phase 2: attention ----
        a2a_in = dram.tile([N_CORES, 128, 512], bf16)
        a2a_out = dram.tile([N_CORES, 128, 512], bf16)

        att_ctx = ExitStack()
        att_sps = att_ctx.enter_context(
            tc.tile_pool(name="att_sps", bufs=1, space="PSUM"))
        att_ops = att_ctx.enter_context(
            tc.tile_pool(name="att_ops", bufs=1, space="PSUM"))
        pp = att_ctx.enter_context(tc.tile_pool(name="pp", bufs=2))
        npool = att_ctx.enter_context(tc.tile_pool(name="npool", bufs=2))
        ndram = att_ctx.enter_context(tc.tile_pool(name="ndram", bufs=2,
                                                   space="DRAM"))

        for b in range(B):
            for h2 in range(2):  # tq half: columns [1024*h2, 1024*h2+1024)
                tq0 = 1024 * h2
                O_ps = [att_ops.tile([128, 1024], f32, tag=f"o{l}",
                                     name=f"O{b}{h2}{l}") for l in range(2)]
                prevP = [None, None]
                for tk in range(16):
                    S_ps = [att_sps.tile([128, 1024], f32, tag=f"s{l}",
                                         name=f"S{b}{h2}{l}_{tk}")
                            for l in range(2)]
                    P_sb = [pp.tile([128, 1024], bf16, tag=f"p{l}",
                                    name=f"P{b}{h2}{l}_{tk}") for l in range(2)]
                    for l in range(2):
                        lo, hi = 64 * l, 64 * (l + 1)
                        for s in range(2):
                            nc.tensor.matmul(
                                S_ps[l][:, 512 * s:512 * (s + 1)],
                                KT_sb[b][lo:hi, 128 * tk:128 * (tk + 1)],
                                QT_sb[b][lo:hi, tq0 + 512 * s:tq0 + 512 * (s + 1)],
                                start=True, stop=True)
                    # V matmuls for the previous tk (softly pipelined)
                    if tk > 0:
                        for l in range(2):
                            for s in range(2):
                                nc.tensor.matmul(
                                    O_ps[l][0:DH + 1, 512 * s:512 * (s + 1)],
                                    V_sb[b][tk - 1][:, l, :],
                                    prevP[l][:, 512 * s:512 * (s + 1)],
                                    start=(tk - 1 == 0), stop=False,
                                    skip_group_check=True)
                    for l in range(2):
                        nc.scalar.activation(P_sb[l][:], S_ps[l][:], AF.Exp,
                                             scale=0.125)
                    prevP = P_sb
                for l in range(2):
                    for s in range(2):
                        nc.tensor.matmul(
                            O_ps[l][0:DH + 1, 512 * s:512 * (s + 1)],
                            V_sb[b][15][:, l, :],
                            prevP[l][:, 512 * s:512 * (s + 1)],
                            start=False, stop=True, skip_group_check=True)
                # normalize + stage for AllToAll
                for l in range(2):
                    recip = npool.tile([1, 1024], f32, tag="recip")
                    nc.vector.reciprocal(recip[:], O_ps[l][DH:DH + 1, :])
                    rd = ndram.tile([1, 1024], f32, tag="rd")
                    nc.sync.dma_start(out=rd[:], in_=recip[:])
                    recipB = npool.tile([64, 1024], f32, tag="recipB")
                    nc.sync.dma_start(out=recipB[:], in_=bcast(rd[0], 64))
                    onorm = npool.tile([64, 1024], bf16, tag="onorm")
                    nc.vector.tensor_mul(onorm[:], O_ps[l][0:DH, :], recipB[:])
                    for s in range(2):
                        j = 4 * b + 2 * h2 + s
                        nc.sync.dma_start(
                            out=a2a_in[j, 64 * l:64 * (l + 1), :],
                            in_=onorm[:, 512 * s:512 * (s + 1)])

        att_ctx.close()

        # ---- phase 3: AllToAll ----
        nc.gpsimd.collective_compute(
            "AllToAll", mybir.AluOpType.bypass,
            replica_groups=[list(range(N_CORES))],
            ins=[a2a_in[:].opt()], outs=[a2a_out[:].opt()])

        og_sb = persist.tile([128, N_CORES, 512], bf16)
        for i in range(N_CORES):
            nc.sync.dma_start(out=og_sb[:, i, :], in_=a2a_out[i])

        # ---- phase 4: projection + residual + LayerNorm ----
        with tc.tile_pool(name="projps", bufs=4, space="PSUM") as pps, \
                tc.tile_pool(name="ln", bufs=3) as ln:
            for mt in range(4):
                y_sb = ln.tile([128, D], f32, tag="y")
                for oc in range(2):
                    ps = pps.tile([128, 512], f32, tag="proj")
                    for g in range(N_CORES):
                        nc.tensor.matmul(
                            ps[:], og_sb[:, g, 128 * mt:128 * (mt + 1)],
                            woT_sb[:, g, 512 * oc:512 * (oc + 1)],
                            start=(g == 0), stop=False, skip_group_check=True)
                    nc.tensor.matmul(
                        ps[:], ones_sb[:],
                        bo_sb[:, 512 * oc:512 * (oc + 1)],
                        start=False, stop=True, skip_group_check=True)
                    # y = proj + residual
                    nc.vector.tensor_add(y_sb[:, 512 * oc:512 * (oc + 1)],
                                         ps[:], xres_sb[:, mt,
                                                        512 * oc:512 * (oc + 1)])
                stats = ln.tile([128, 2, nc.vector.BN_STATS_DIM], f32,
                                tag="stats")
                for sg in range(2):
                    nc.vector.bn_stats(out=stats[:, sg, :],
                                       in_=y_sb[:, 512 * sg:512 * (sg + 1)])
                mv = ln.tile([128, nc.vector.BN_AGGR_DIM], f32, tag="mv")
                nc.vector.bn_aggr(out=mv[:], in_=stats[:])
                std = ln.tile([128, 1], f32, tag="std")
                nc.scalar.activation(std[:], mv[:, 1:2], AF.Sqrt,
                                     bias=eps_sb[:])
                rstd = ln.tile([128, 1], f32, tag="rstd")
                nc.vector.reciprocal(rstd[:], std[:])
                yn = ln.tile([128, D], f32, tag="yn")
                nc.vector.tensor_scalar(yn[:], y_sb[:], mv[:, 0:1], rstd[:],
                                        ALU.subtract, ALU.mult)
                fin = ln.tile([128, D], f32, tag="fin")
                nc.vector.scalar_tensor_tensor(fin[:], yn[:], 1.0, gamma_sb[:],
                                               ALU.mult, ALU.mult)
                nc.vector.tensor_add(fin[:], fin[:], beta_sb[:])
                nc.sync.dma_start(out=out_d[128 * mt:128 * (mt + 1), :],
                                  in_=fin[:])

    nc.compile()
    return nc


def _prep_inputs(x, Wq, bq, Wk, bk, Wv, bv, Wo, bo, gamma, beta):
    x = np.asarray(x, dtype=np.float32)
    Wq, Wk, Wv, Wo = (np.asarray(a, dtype=np.float32) for a in (Wq, Wk, Wv, Wo))
    bq, bk, bv, bo = (np.asarray(a, dtype=np.float32) for a in (bq, bk, bv, bo))
    gamma = np.asarray(gamma, dtype=np.float32)
    beta = np.asarray(beta, dtype=np.float32)

    xT = [np.ascontiguousarray(x[b].T).astype(BF16) for b in range(B)]
    # gathered channel order (same for every core): src-core-major
    ch_gath = np.empty(D, dtype=np.int64)
    for g in range(D):
        i, jj, dd = g // 128, (g % 128) // 64, g % 64
        ch_gath[g] = 16 * dd + (2 * i + jj)
    woT = np.ascontiguousarray(Wo[:, ch_gath].T).astype(BF16)
    bo_bf = bo.astype(BF16)

    in_maps = []
    for c in range(N_CORES):
        ch_loc = np.empty(128, dtype=np.int64)
        for g in range(128):
            jj, dd = g // 64, g % 64
            ch_loc[g] = 16 * dd + (2 * c + jj)
        bi, rb = c // 4, c % 4
        m = {
            "xT0": xT[0], "xT1": xT[1],
            "wqT": np.ascontiguousarray(Wq[ch_loc, :].T).astype(BF16),
            "wkT": np.ascontiguousarray(Wk[ch_loc, :].T).astype(BF16),
            "wvT": np.ascontiguousarray(Wv[ch_loc, :].T).astype(BF16),
            "bq": np.ascontiguousarray(bq[ch_loc]),
            "bk": np.ascontiguousarray(bk[ch_loc]),
            "bv": np.ascontiguousarray(bv[ch_loc]),
            "woT": woT, "bo": bo_bf, "gamma": gamma, "beta": beta,
            "xres": np.ascontiguousarray(x[bi, 512 * rb:512 * (rb + 1), :]),
        }
        in_maps.append(m)
    return in_maps


def _run(in_maps):
    from concourse.bass_utils import run_bass_kernel_spmd
    if "nc" not in _CACHE:
        _CACHE["nc"] = _build()
    res = run_bass_kernel_spmd(_CACHE["nc"], in_maps,
                               core_ids=list(range(N_CORES)))
    return res


def kernel(x, Wq, bq, Wk, bk, Wv, bv, Wo, bo, gamma, beta):
    in_maps = _prep_inputs(x, Wq, bq, Wk, bk, Wv, bv, Wo, bo, gamma, beta)
    res = _run(in_maps)
    out = np.empty((B, T, D), dtype=np.float32)
    for c in range(N_CORES):
        bi, rb = c // 4, c % 4
        out[bi, 512 * rb:512 * (rb + 1), :] = res.results[c]["out"]
    return out
